# revision 38
# baseline (speedup 1.0000x reference)
"""Distributed contrastive-loss kernel for one TRN2 chip (8 NeuronCores).

loss = mean_i( logsumexp_j(l_ij) - l_{i,t_i} ),  l = (a_hat @ c_hat.T) / tau

Sharding: data-parallel over anchor rows (N/8 = 2048 per core); candidates are
replicated to every core; per-row NLL comes back and the host takes the mean.

Per-core pipeline (~319 us HW; ScalarE's ~1 elem/ns exp stream is the floor):
  - A-prep: batched loads, row sum-of-squares, Newton rsqrt on DVE (const
    seed, inputs ~ chi2(D)), then anchors are scaled by ra/tau during the
    bf16 cast, so PSUM accumulates FINAL logits. A^T built by TensorE
    transposes (PSUM is idle in the head).
  - C-prep (per 2048-row group): batched split loads, f32->bf16 cast on
    GPSIMD, row norms on DVE from bf16, Newton rsqrt, normalize via 4x-mode
    bf16 tensor_scalar. Group 0 transposes on TensorE (shortest head);
    groups 1+ go bf16 rows -> DRAM scratch -> DMA-xbar transposed loads.
  - Main loop: bf16 matmuls (K=256 as two 128-partition halves) into
    [128, 2048] f32 PSUM spans (4 banks, double-buffered). Sum-of-exp per
    span: 3 of 4 spans on ScalarE (exact Exp + accum_out); 1 of 4 via the
    Schraudolph bitcast exp pipelined DVE->GPSIMD->DVE: DVE copies the span
    to bf16 SBUF, GPSIMD computes i32(x*S + B) with immediate scalars, DVE
    reduces the i32 tile reinterpreted as f32 (B calibrated so the expected
    sum-of-exp ratio is 1).
  - Finalize: reduce partials, Ln on ScalarE, nll = lse - dot*rtc (the dot
    against target-candidate rows already carries ra/tau via the scaled a).

The logits are bounded (~N(0,0.9), |l| <= 14.3), so exp needs no
max-subtraction in f32.
"""

import numpy as np

import concourse.bass as bass
import concourse.mybir as mybir
from concourse import bacc, tile, masks
from concourse.bass_utils import run_bass_kernel_spmd

F32 = mybir.dt.float32
BF16 = mybir.dt.bfloat16
I32 = mybir.dt.int32
ALU = mybir.AluOpType
ACTF = mybir.ActivationFunctionType

N_CORES = 8
N_FULL = 16384
M_FULL = 16384
D = 256
TAU = 0.07

SCHRAUDOLPH_S = float(2 ** 23 / np.log(2))
SCHRAUDOLPH_B = 1064870532.413013   # calibrated: E[sum approx / sum exact] = 1
GPS_SPAN_MOD = 1000000                    # span k offloaded iff k % MOD == MOD-1


def _emit_rsqrt(nc, pool, x_ap, w, seed, iters=4):
    """Newton rsqrt on DVE: y' = y*(1.5 - 0.5*x*y^2), const seed.

    Inputs are sums of squares of D-dim randn rows, concentrated around D,
    so the constant seed 1/sqrt(D) converges in <=4 iterations.
    """
    y0 = pool.tile([128, w], F32, tag="nwt_y0")
    nc.vector.memset(y0[:], seed)
    y = y0[:]
    for _ in range(iters):
        t = pool.tile([128, w], F32, tag="nwt_t")
        nc.vector.tensor_mul(t[:], y, y)
        t2 = pool.tile([128, w], F32, tag="nwt_t2")
        nc.vector.scalar_tensor_tensor(t2[:], t[:], -0.5, x_ap, op0=ALU.mult, op1=ALU.mult)
        y2 = pool.tile([128, w], F32, tag="nwt_y2")
        nc.vector.scalar_tensor_tensor(y2[:], t2[:], 1.5, y, op0=ALU.add, op1=ALU.mult)
        y = y2[:]
    return y


def build_graph(NL=N_FULL // N_CORES, M=M_FULL, MGW=2048, num_devices=N_CORES):
    """Build + compile the per-core Bass graph. All cores run the same graph."""
    NT = NL // 128         # anchor tiles per core
    MG = M // MGW          # candidate column groups
    CTG = MGW // 128       # candidate row-tiles per group
    SPW = min(2048, MGW)   # exp span width (4 PSUM banks)
    SP = MGW // SPW        # spans per (group, n-tile)
    NSC = SPW // 512       # 512-wide matmul chunks per span
    NQ = 4                 # split factor for the big input loads

    nc = bacc.Bacc("TRN2", target_bir_lowering=False, debug=False,
                   num_devices=num_devices)

    anch = nc.dram_tensor("anch", [NL, D], F32, kind="ExternalInput")
    cand = nc.dram_tensor("cand", [M, D], F32, kind="ExternalInput")
    tcand = nc.dram_tensor("tcand", [NL, D], F32, kind="ExternalInput")
    nll_out = nc.dram_tensor("nll", [128, NT], F32, kind="ExternalOutput")

    with tile.TileContext(nc) as tc:
        with (
            tc.tile_pool(name="persist", bufs=1) as persist,
            tc.tile_pool(name="cspan", bufs=2) as cspan_pool,
            tc.tile_pool(name="cbfp", bufs=2) as cbf_pool,
            tc.tile_pool(name="etrash", bufs=2) as etrash_pool,
            tc.tile_pool(name="small", bufs=2) as small,
            tc.tile_pool(name="nwt", bufs=2) as nwt,
            tc.tile_pool(name="dram", bufs=1, space="DRAM") as dram,
            tc.tile_pool(name="psum", bufs=2, space="PSUM") as psum,
        ):
            abf = persist.tile([128, NT * D], BF16, tag="abf")
            at = persist.tile([128, 2 * NL], BF16, tag="at")
            cts = [persist.tile([128, 2 * MGW], BF16, tag=f"ct{g}", name=f"ct{g}")
                   for g in range(MG)]
            ident = persist.tile([128, 128], BF16, tag="ident")
            ones_sc = persist.tile([128, 1], F32, tag="ones_sc")
            anormsq = persist.tile([128, NT], F32, tag="anormsq")
            ra_tau = persist.tile([128, NT], F32, tag="ra_tau")
            ra_tau_s = persist.tile([128, NT], F32, tag="ra_tau_s")
            tnormsq = persist.tile([128, NT], F32, tag="tnormsq")
            tdot = persist.tile([128, NT], F32, tag="tdot")
            ltgt = persist.tile([128, NT], F32, tag="ltgt")
            separts = persist.tile([128, NT * MG * SP], F32, tag="separts")
            sumexp = persist.tile([128, NT], F32, tag="sumexp")
            lse = persist.tile([128, NT], F32, tag="lse")
            nll_sb = persist.tile([128, NT], F32, tag="nll_sb")

            scr_c = [dram.tile([MGW, D], BF16, tag=f"scr_c{g}", name=f"scr_c{g}")
                     for g in range(1, MG)]

            trash_pool = small  # [128, D] trash targets for accum-only ops

            masks.make_identity(nc, ident[:])
            nc.vector.memset(ones_sc[:], 1.0)

            def split_load(dst_span, src, rows0, ntiles):
                """Load [ntiles*128, D] rows of src into dst_span [128, ntiles*D],
                split into NQ parallel sub-DMAs."""
                per = max(1, ntiles // NQ)
                for q0 in range(0, ntiles, per):
                    q1 = min(q0 + per, ntiles)
                    nc.sync.dma_start(
                        dst_span[:, q0 * D:q1 * D]
                        .rearrange("p (j d) -> p j d", d=D),
                        src[rows0 + q0 * 128: rows0 + q1 * 128, :]
                        .rearrange("(j p) d -> p j d", p=128))

            def pe_transpose_to(dst, src_bf, ntiles):
                """dst [128, 2*ntiles*128] (d-major halves) <- transpose of
                src_bf [128, ntiles*D] via TensorE + one DVE copy."""
                ptr = psum.tile([128, 2 * ntiles * 128], BF16, tag="pm",
                                name=f"ptr_{dst.tensor.name}")
                for h in range(2):
                    for j in range(ntiles):
                        nc.tensor.transpose(
                            ptr[:, (h * ntiles + j) * 128:(h * ntiles + j + 1) * 128],
                            src_bf[:, j * D + h * 128: j * D + h * 128 + 128],
                            ident[:])
                nc.vector.tensor_copy(dst[:], ptr[:])

            NPQ = 4     # prep pipeline quarters per group

            def cprep_state(g):
                return {
                    "cspan": cspan_pool.tile([128, CTG * D], F32, tag="cspan",
                                             name=f"cspan{g}"),
                    "craw": cbf_pool.tile([128, CTG * D], BF16, tag="craw",
                                          name=f"craw{g}"),
                    "cns": small.tile([128, CTG], F32, tag="cns", name=f"cns{g}"),
                    "cbf": cbf_pool.tile([128, CTG * D], BF16, tag="cbf",
                                         name=f"cbf{g}"),
                }

            def cprep_part(g, st, pq):
                qt = CTG // NPQ
                j0 = pq * qt
                cspan, craw, cns = st["cspan"], st["craw"], st["cns"]
                nc.sync.dma_start(
                    cspan[:, j0 * D:(j0 + qt) * D]
                    .rearrange("p (j d) -> p j d", d=D),
                    cand[g * MGW + j0 * 128: g * MGW + (j0 + qt) * 128, :]
                    .rearrange("(j p) d -> p j d", p=128))
                nc.vector.tensor_copy(craw[:, j0 * D:(j0 + qt) * D],
                                      cspan[:, j0 * D:(j0 + qt) * D])
                for j in range(j0, j0 + qt):
                    sl = craw[:, j * D:(j + 1) * D]
                    tr = trash_pool.tile([128, D], BF16, tag="trashb",
                                         name=f"tr{g}_{j}")
                    nc.vector.scalar_tensor_tensor(
                        tr[:], sl, 0.0, sl, op0=ALU.bypass, op1=ALU.mult,
                        accum_out=cns[:, j:j + 1])

            def cprep_scales(g, st):
                craw, cns, cbf = st["craw"], st["cns"], st["cbf"]
                rc = _emit_rsqrt(nc, nwt, cns[:], CTG, seed=D ** -0.5)
                for j in range(CTG):
                    nc.vector.tensor_scalar(
                        cbf[:, j * D:(j + 1) * D], craw[:, j * D:(j + 1) * D],
                        rc[:, j:j + 1], None, op0=ALU.mult)

            def cprep_finish(g, st):
                cbf = st["cbf"]
                if g == 0:
                    pe_transpose_to(cts[0], cbf, CTG)
                else:
                    nc.gpsimd.dma_start(
                        scr_c[g - 1][:, :].rearrange("(j p) d -> p j d", p=128),
                        cbf[:].rearrange("p (j d) -> p j d", d=D))
                    for h in range(2):
                        nc.sync.dma_start(cts[g][:, h * MGW:(h + 1) * MGW],
                                          scr_c[g - 1][:, h * 128:(h + 1) * 128],
                                          transpose=True)

            def emit_cprep(g):
                st = cprep_state(g)
                for pq in range(NPQ):
                    cprep_part(g, st, pq)
                cprep_scales(g, st)
                cprep_finish(g, st)

            # ---- group 0 C-prep first (head critical path) ----
            emit_cprep(0)

            # ---- A-prep: cast + transpose critical; norms/ra in parallel ----
            a_span = cspan_pool.tile([128, NT * D], F32, tag="cspan",
                                     name="a_span")
            aqt = max(1, NT // 4)
            for t0 in range(0, NT, aqt):
                nc.sync.dma_start(
                    a_span[:, t0 * D:(t0 + aqt) * D]
                    .rearrange("p (j d) -> p j d", d=D),
                    anch[t0 * 128:(t0 + aqt) * 128, :]
                    .rearrange("(j p) d -> p j d", p=128))
                nc.vector.tensor_copy(abf[:, t0 * D:(t0 + aqt) * D],
                                      a_span[:, t0 * D:(t0 + aqt) * D])
            pe_transpose_to(at, abf, NT)
            for t in range(NT):
                sl = a_span[:, t * D:(t + 1) * D]
                tr = trash_pool.tile([128, D], BF16, tag="trashb", name=f"tra{t}")
                nc.scalar.activation(tr[:], sl, ACTF.Square,
                                     accum_out=anormsq[:, t:t + 1])
            ra = _emit_rsqrt(nc, nwt, anormsq[:], NT, seed=D ** -0.5)
            nc.vector.tensor_scalar_mul(ra_tau[:], ra, 1.0 / TAU)
            nc.vector.tensor_scalar_mul(ra_tau_s[:], ra_tau[:], SCHRAUDOLPH_S)

            # ---- prefetch C-prep for group 1 (rest interleave below) ----
            if MG > 1:
                emit_cprep(1)

            # ---- prep task queue: C-groups and the target-logit path ----
            from collections import deque
            tasks = deque()

            def queue_group(g):
                st = cprep_state(g)
                for pq in range(NPQ):
                    tasks.append((g, lambda g=g, st=st, pq=pq: cprep_part(g, st, pq)))
                tasks.append((g, lambda g=g, st=st: cprep_scales(g, st)))
                tasks.append((g, lambda g=g, st=st: cprep_finish(g, st)))

            def tc_task(q):
                qt = max(1, NT // 4)
                t0 = q * qt
                if t0 >= NT:
                    return
                tc_span = tc_spans[0]
                nc.sync.dma_start(
                    tc_span[:, t0 * D:(t0 + qt) * D]
                    .rearrange("p (j d) -> p j d", d=D),
                    tcand[t0 * 128:(t0 + qt) * 128, :]
                    .rearrange("(j p) d -> p j d", p=128))
                for t in range(t0, t0 + qt):
                    tsl = tc_span[:, t * D:(t + 1) * D]
                    tr = trash_pool.tile([128, D], F32, tag="trash", name=f"trt{t}")
                    nc.vector.scalar_tensor_tensor(
                        tr[:], tsl, 0.0, tsl, op0=ALU.bypass, op1=ALU.mult,
                        accum_out=tnormsq[:, t:t + 1])
                    tr2 = trash_pool.tile([128, D], F32, tag="trash", name=f"trd{t}")
                    nc.vector.scalar_tensor_tensor(
                        tr2[:], a_span[:, t * D:(t + 1) * D], 0.0, tsl,
                        op0=ALU.bypass, op1=ALU.mult,
                        accum_out=tdot[:, t:t + 1])

            def tc_finish():
                rtc = _emit_rsqrt(nc, nwt, tnormsq[:], NT, seed=D ** -0.5)
                tmp2 = small.tile([128, NT], F32, tag="ltg2")
                nc.vector.tensor_mul(tmp2[:], tdot[:], ra_tau[:])
                nc.vector.tensor_mul(ltgt[:], tmp2[:], rtc)

            tc_spans = [cspan_pool.tile([128, NT * D], F32, tag="cspan",
                                        name="tc_span")]
            def queue_tc():
                for q in range(4):
                    tasks.append((None, lambda q=q: tc_task(q)))
                tasks.append((None, tc_finish))

            tc_queued = False
            for g in range(2, MG):
                queue_group(g)
                if g == 3:
                    queue_tc()
                    tc_queued = True
            if not tc_queued:
                queue_tc()

            # ---- main loop (prep tasks drip between span groups) ----
            span_idx = 0
            for g in range(MG):
                while tasks and tasks[0][0] is not None and tasks[0][0] <= g + 1:
                    tasks.popleft()[1]()
                for t in range(NT):
                    if tasks:
                        tasks.popleft()[1]()
                    for hg in range(SP):
                        pm = psum.tile([128, SPW], F32, tag="pm",
                                       name=f"pm{g}_{t}_{hg}")
                        for h in range(2):
                            lhsT = at[:, h * NL + t * 128: h * NL + (t + 1) * 128]
                            for sc in range(NSC):
                                col = h * MGW + hg * SPW + sc * 512
                                nc.tensor.matmul(
                                    pm[:, sc * 512:(sc + 1) * 512],
                                    lhsT=lhsT,
                                    rhs=cts[g][:, col:col + 512],
                                    start=(h == 0), stop=(h == 1))
                        k = (t * MG + g) * SP + hg
                        if span_idx % GPS_SPAN_MOD == GPS_SPAN_MOD - 1:
                            ei = etrash_pool.tile([128, SPW], I32, tag="ei",
                                                  name=f"ei{k}")
                            nc.vector.tensor_scalar(
                                ei[:], pm[:], ra_tau_s[:, t:t + 1], SCHRAUDOLPH_B,
                                op0=ALU.mult, op1=ALU.add)
                            erb = etrash_pool.tile([128, SPW], BF16, tag="erb",
                                                   name=f"erb{k}")
                            nc.vector.tensor_scalar(
                                erb[:], ei[:].bitcast(F32), 1.0, None,
                                op0=ALU.mult, op1=ALU.add,
                                accum_out=separts[:, k:k + 1])
                        else:
                            etr = etrash_pool.tile([128, SPW], BF16, tag="etr",
                                                   name=f"etr{k}")
                            nc.scalar.activation(
                                etr[:], pm[:], ACTF.Exp, scale=ra_tau[:, t:t + 1],
                                accum_out=separts[:, k:k + 1])
                        span_idx += 1

            while tasks:
                tasks.popleft()[1]()

            # ---- finalize ----
            nc.vector.reduce_sum(
                sumexp[:],
                separts[:].rearrange("p (t r) -> p t r", t=NT),
                axis=mybir.AxisListType.X)
            nc.scalar.activation(lse[:], sumexp[:], ACTF.Ln)
            nc.vector.tensor_sub(nll_sb[:], lse[:], ltgt[:])
            nc.gpsimd.dma_start(nll_out[:, :], nll_sb[:])

    nc.compile()
    return nc


_CACHE = {}


def _compiled():
    if "nc" not in _CACHE:
        _CACHE["nc"] = build_graph()
    return _CACHE["nc"]


def make_in_maps(anchors, candidates, targets):
    anchors = np.ascontiguousarray(np.asarray(anchors, dtype=np.float32))
    candidates = np.ascontiguousarray(np.asarray(candidates, dtype=np.float32))
    targets = np.asarray(targets, dtype=np.int32)
    tc_full = candidates[targets]          # [N, D] host gather of target rows
    nl = anchors.shape[0] // N_CORES
    in_maps = []
    for c in range(N_CORES):
        sl = slice(c * nl, (c + 1) * nl)
        in_maps.append({
            "anch": np.ascontiguousarray(anchors[sl]),
            "cand": candidates,
            "tcand": np.ascontiguousarray(tc_full[sl]),
        })
    return in_maps


def kernel(anchors, candidates, targets):
    nc = _compiled()
    in_maps = make_in_maps(anchors, candidates, targets)
    res = run_bass_kernel_spmd(nc, in_maps, core_ids=list(range(N_CORES)))
    nll = np.stack([np.asarray(r["nll"], dtype=np.float64) for r in res.results])
    return np.float32(nll.mean())


# revision 39
# speedup vs baseline: 1.0017x; 1.0017x over previous
"""Distributed contrastive-loss kernel for one TRN2 chip (8 NeuronCores).

loss = mean_i( logsumexp_j(l_ij) - l_{i,t_i} ),  l = (a_hat @ c_hat.T) / tau

Sharding: data-parallel over anchor rows (N/8 = 2048 per core); candidates are
replicated to every core; per-row NLL comes back and the host takes the mean.

Per-core pipeline (~319 us HW; ScalarE's ~1 elem/ns exp stream is the floor):
  - A-prep: batched loads, row sum-of-squares, Newton rsqrt on DVE (const
    seed, inputs ~ chi2(D)), then anchors are scaled by ra/tau during the
    bf16 cast, so PSUM accumulates FINAL logits. A^T built by TensorE
    transposes (PSUM is idle in the head).
  - C-prep (per 2048-row group): batched split loads, f32->bf16 cast on
    DVE, row norms via scalar_tensor_tensor accum_out, Newton rsqrt,
    normalize via 4x-mode bf16 tensor_scalar, emitted as fine-grained tasks
    dripped between main-loop iterations. Group 0 transposes on TensorE
    (shortest head); groups 1+ go bf16 rows -> DRAM scratch (SWDGE) ->
    DMA-xbar transposed loads.
  - Main loop: bf16 matmuls (K=256 as two 128-partition halves) into
    [128, 2048] f32 PSUM spans (4 banks, double-buffered), one ScalarE Exp
    per span (AP scale, accum_out row-sums). A single PSUM consumer keeps
    the rotation gap-free; every measured offload of spans to a second
    engine (DVE/GPSIMD Schraudolph, disabled via GPS_SPAN_MOD) lost ~2.5us
    per consumer switch.
  - Finalize: reduce partials, Ln on ScalarE, nll = lse - dot*rtc (the dot
    against target-candidate rows already carries ra/tau via the scaled a).

The logits are bounded (~N(0,0.9), |l| <= 14.3), so exp needs no
max-subtraction in f32.
"""

import numpy as np

import concourse.bass as bass
import concourse.mybir as mybir
from concourse import bacc, tile, masks
from concourse.bass_utils import run_bass_kernel_spmd

F32 = mybir.dt.float32
BF16 = mybir.dt.bfloat16
I32 = mybir.dt.int32
ALU = mybir.AluOpType
ACTF = mybir.ActivationFunctionType

N_CORES = 8
N_FULL = 16384
M_FULL = 16384
D = 256
TAU = 0.07

SCHRAUDOLPH_S = float(2 ** 23 / np.log(2))
SCHRAUDOLPH_B = 1064870532.413013   # calibrated: E[sum approx / sum exact] = 1
GPS_SPAN_MOD = 1000000                    # span k offloaded iff k % MOD == MOD-1


def _emit_rsqrt(nc, pool, x_ap, w, seed, iters=4):
    """Newton rsqrt on DVE: y' = y*(1.5 - 0.5*x*y^2), const seed.

    Inputs are sums of squares of D-dim randn rows, concentrated around D,
    so the constant seed 1/sqrt(D) converges in <=4 iterations.
    """
    y0 = pool.tile([128, w], F32, tag="nwt_y0")
    nc.vector.memset(y0[:], seed)
    y = y0[:]
    for _ in range(iters):
        t = pool.tile([128, w], F32, tag="nwt_t")
        nc.vector.tensor_mul(t[:], y, y)
        t2 = pool.tile([128, w], F32, tag="nwt_t2")
        nc.vector.scalar_tensor_tensor(t2[:], t[:], -0.5, x_ap, op0=ALU.mult, op1=ALU.mult)
        y2 = pool.tile([128, w], F32, tag="nwt_y2")
        nc.vector.scalar_tensor_tensor(y2[:], t2[:], 1.5, y, op0=ALU.add, op1=ALU.mult)
        y = y2[:]
    return y


def build_graph(NL=N_FULL // N_CORES, M=M_FULL, MGW=2048, num_devices=N_CORES):
    """Build + compile the per-core Bass graph. All cores run the same graph."""
    NT = NL // 128         # anchor tiles per core
    MG = M // MGW          # candidate column groups
    CTG = MGW // 128       # candidate row-tiles per group
    SPW = min(2048, MGW)   # exp span width (4 PSUM banks)
    SP = MGW // SPW        # spans per (group, n-tile)
    NSC = SPW // 512       # 512-wide matmul chunks per span
    NQ = 4                 # split factor for the big input loads

    nc = bacc.Bacc("TRN2", target_bir_lowering=False, debug=False,
                   num_devices=num_devices)

    anch = nc.dram_tensor("anch", [NL, D], F32, kind="ExternalInput")
    cand = nc.dram_tensor("cand", [M, D], F32, kind="ExternalInput")
    tcand = nc.dram_tensor("tcand", [NL, D], F32, kind="ExternalInput")
    nll_out = nc.dram_tensor("nll", [128, NT], F32, kind="ExternalOutput")

    with tile.TileContext(nc) as tc:
        with (
            tc.tile_pool(name="persist", bufs=1) as persist,
            tc.tile_pool(name="cspan", bufs=2) as cspan_pool,
            tc.tile_pool(name="cbfp", bufs=2) as cbf_pool,
            tc.tile_pool(name="etrash", bufs=2) as etrash_pool,
            tc.tile_pool(name="small", bufs=2) as small,
            tc.tile_pool(name="nwt", bufs=2) as nwt,
            tc.tile_pool(name="dram", bufs=1, space="DRAM") as dram,
            tc.tile_pool(name="psum", bufs=2, space="PSUM") as psum,
        ):
            abf = persist.tile([128, NT * D], BF16, tag="abf")
            at = persist.tile([128, 2 * NL], BF16, tag="at")
            cts = [persist.tile([128, 2 * MGW], BF16, tag=f"ct{g}", name=f"ct{g}")
                   for g in range(MG)]
            ident = persist.tile([128, 128], BF16, tag="ident")
            ones_sc = persist.tile([128, 1], F32, tag="ones_sc")
            anormsq = persist.tile([128, NT], F32, tag="anormsq")
            ra_tau = persist.tile([128, NT], F32, tag="ra_tau")
            ra_tau_s = persist.tile([128, NT], F32, tag="ra_tau_s")
            tnormsq = persist.tile([128, NT], F32, tag="tnormsq")
            tdot = persist.tile([128, NT], F32, tag="tdot")
            ltgt = persist.tile([128, NT], F32, tag="ltgt")
            separts = persist.tile([128, NT * MG * SP], F32, tag="separts")
            sumexp = persist.tile([128, NT], F32, tag="sumexp")
            lse = persist.tile([128, NT], F32, tag="lse")
            nll_sb = persist.tile([128, NT], F32, tag="nll_sb")

            scr_c = [dram.tile([MGW, D], BF16, tag=f"scr_c{g}", name=f"scr_c{g}")
                     for g in range(1, MG)]

            trash_pool = small  # [128, D] trash targets for accum-only ops

            masks.make_identity(nc, ident[:])
            nc.vector.memset(ones_sc[:], 1.0)

            def split_load(dst_span, src, rows0, ntiles):
                """Load [ntiles*128, D] rows of src into dst_span [128, ntiles*D],
                split into NQ parallel sub-DMAs."""
                per = max(1, ntiles // NQ)
                for q0 in range(0, ntiles, per):
                    q1 = min(q0 + per, ntiles)
                    nc.sync.dma_start(
                        dst_span[:, q0 * D:q1 * D]
                        .rearrange("p (j d) -> p j d", d=D),
                        src[rows0 + q0 * 128: rows0 + q1 * 128, :]
                        .rearrange("(j p) d -> p j d", p=128))

            def pe_transpose_to(dst, src_bf, ntiles):
                """dst [128, 2*ntiles*128] (d-major halves) <- transpose of
                src_bf [128, ntiles*D] via TensorE + one DVE copy."""
                ptr = psum.tile([128, 2 * ntiles * 128], BF16, tag="pm",
                                name=f"ptr_{dst.tensor.name}")
                for h in range(2):
                    for j in range(ntiles):
                        nc.tensor.transpose(
                            ptr[:, (h * ntiles + j) * 128:(h * ntiles + j + 1) * 128],
                            src_bf[:, j * D + h * 128: j * D + h * 128 + 128],
                            ident[:])
                nc.vector.tensor_copy(dst[:], ptr[:])

            NPQ = 4     # prep pipeline quarters per group

            def cprep_state(g):
                return {
                    "cspan": cspan_pool.tile([128, CTG * D], F32, tag="cspan",
                                             name=f"cspan{g}"),
                    "craw": cbf_pool.tile([128, CTG * D], BF16, tag="craw",
                                          name=f"craw{g}"),
                    "cns": small.tile([128, CTG], F32, tag="cns", name=f"cns{g}"),
                    "cbf": cbf_pool.tile([128, CTG * D], BF16, tag="cbf",
                                         name=f"cbf{g}"),
                }

            def cprep_part(g, st, pq):
                qt = CTG // NPQ
                j0 = pq * qt
                cspan, craw, cns = st["cspan"], st["craw"], st["cns"]
                nc.sync.dma_start(
                    cspan[:, j0 * D:(j0 + qt) * D]
                    .rearrange("p (j d) -> p j d", d=D),
                    cand[g * MGW + j0 * 128: g * MGW + (j0 + qt) * 128, :]
                    .rearrange("(j p) d -> p j d", p=128))
                nc.vector.tensor_copy(craw[:, j0 * D:(j0 + qt) * D],
                                      cspan[:, j0 * D:(j0 + qt) * D])
                for j in range(j0, j0 + qt):
                    sl = craw[:, j * D:(j + 1) * D]
                    tr = trash_pool.tile([128, D], BF16, tag="trashb",
                                         name=f"tr{g}_{j}")
                    nc.vector.scalar_tensor_tensor(
                        tr[:], sl, 0.0, sl, op0=ALU.bypass, op1=ALU.mult,
                        accum_out=cns[:, j:j + 1])

            def cprep_scales(g, st):
                craw, cns, cbf = st["craw"], st["cns"], st["cbf"]
                rc = _emit_rsqrt(nc, nwt, cns[:], CTG, seed=D ** -0.5)
                for j in range(CTG):
                    nc.vector.tensor_scalar(
                        cbf[:, j * D:(j + 1) * D], craw[:, j * D:(j + 1) * D],
                        rc[:, j:j + 1], None, op0=ALU.mult)

            def cprep_finish(g, st):
                cbf = st["cbf"]
                if g == 0:
                    pe_transpose_to(cts[0], cbf, CTG)
                else:
                    nc.gpsimd.dma_start(
                        scr_c[g - 1][:, :].rearrange("(j p) d -> p j d", p=128),
                        cbf[:].rearrange("p (j d) -> p j d", d=D))
                    for h in range(2):
                        nc.sync.dma_start(cts[g][:, h * MGW:(h + 1) * MGW],
                                          scr_c[g - 1][:, h * 128:(h + 1) * 128],
                                          transpose=True)

            def emit_cprep(g):
                st = cprep_state(g)
                for pq in range(NPQ):
                    cprep_part(g, st, pq)
                cprep_scales(g, st)
                cprep_finish(g, st)

            # ---- group 0 C-prep first (head critical path) ----
            emit_cprep(0)

            # ---- A-prep: cast + transpose critical; norms/ra in parallel ----
            a_span = cspan_pool.tile([128, NT * D], F32, tag="cspan",
                                     name="a_span")
            aqt = max(1, NT // 4)
            for t0 in range(0, NT, aqt):
                nc.sync.dma_start(
                    a_span[:, t0 * D:(t0 + aqt) * D]
                    .rearrange("p (j d) -> p j d", d=D),
                    anch[t0 * 128:(t0 + aqt) * 128, :]
                    .rearrange("(j p) d -> p j d", p=128))
                nc.vector.tensor_copy(abf[:, t0 * D:(t0 + aqt) * D],
                                      a_span[:, t0 * D:(t0 + aqt) * D])
            pe_transpose_to(at, abf, NT)
            for t in range(NT):
                sl = a_span[:, t * D:(t + 1) * D]
                tr = trash_pool.tile([128, D], BF16, tag="trashb", name=f"tra{t}")
                nc.scalar.activation(tr[:], sl, ACTF.Square,
                                     accum_out=anormsq[:, t:t + 1])
            ra = _emit_rsqrt(nc, nwt, anormsq[:], NT, seed=D ** -0.5)
            nc.vector.tensor_scalar_mul(ra_tau[:], ra, 1.0 / TAU)
            nc.vector.tensor_scalar_mul(ra_tau_s[:], ra_tau[:], SCHRAUDOLPH_S)

            # ---- prefetch C-prep for group 1 (rest interleave below) ----
            if MG > 1:
                emit_cprep(1)

            # ---- prep task queue: C-groups and the target-logit path ----
            from collections import deque
            tasks = deque()

            def queue_group(g):
                st = cprep_state(g)
                for pq in range(NPQ):
                    tasks.append((g, lambda g=g, st=st, pq=pq: cprep_part(g, st, pq)))
                tasks.append((g, lambda g=g, st=st: cprep_scales(g, st)))
                tasks.append((g, lambda g=g, st=st: cprep_finish(g, st)))

            def tc_task(q):
                qt = max(1, NT // 4)
                t0 = q * qt
                if t0 >= NT:
                    return
                tc_span = tc_spans[0]
                nc.sync.dma_start(
                    tc_span[:, t0 * D:(t0 + qt) * D]
                    .rearrange("p (j d) -> p j d", d=D),
                    tcand[t0 * 128:(t0 + qt) * 128, :]
                    .rearrange("(j p) d -> p j d", p=128))
                for t in range(t0, t0 + qt):
                    tsl = tc_span[:, t * D:(t + 1) * D]
                    tr = trash_pool.tile([128, D], F32, tag="trash", name=f"trt{t}")
                    nc.vector.scalar_tensor_tensor(
                        tr[:], tsl, 0.0, tsl, op0=ALU.bypass, op1=ALU.mult,
                        accum_out=tnormsq[:, t:t + 1])
                    tr2 = trash_pool.tile([128, D], F32, tag="trash", name=f"trd{t}")
                    nc.vector.scalar_tensor_tensor(
                        tr2[:], a_span[:, t * D:(t + 1) * D], 0.0, tsl,
                        op0=ALU.bypass, op1=ALU.mult,
                        accum_out=tdot[:, t:t + 1])

            def tc_finish():
                rtc = _emit_rsqrt(nc, nwt, tnormsq[:], NT, seed=D ** -0.5)
                tmp2 = small.tile([128, NT], F32, tag="ltg2")
                nc.vector.tensor_mul(tmp2[:], tdot[:], ra_tau[:])
                nc.vector.tensor_mul(ltgt[:], tmp2[:], rtc)

            tc_spans = [cspan_pool.tile([128, NT * D], F32, tag="cspan",
                                        name="tc_span")]
            def queue_tc():
                for q in range(4):
                    tasks.append((None, lambda q=q: tc_task(q)))
                tasks.append((None, tc_finish))

            tc_queued = False
            for g in range(2, MG):
                queue_group(g)
                if g == 3:
                    queue_tc()
                    tc_queued = True
            if not tc_queued:
                queue_tc()

            # ---- main loop (prep tasks drip between span groups) ----
            span_idx = 0
            for g in range(MG):
                while tasks and tasks[0][0] is not None and tasks[0][0] <= g + 1:
                    tasks.popleft()[1]()
                for t in range(NT):
                    if tasks:
                        tasks.popleft()[1]()
                    for hg in range(SP):
                        pm = psum.tile([128, SPW], F32, tag="pm",
                                       name=f"pm{g}_{t}_{hg}")
                        for h in range(2):
                            lhsT = at[:, h * NL + t * 128: h * NL + (t + 1) * 128]
                            for sc in range(NSC):
                                col = h * MGW + hg * SPW + sc * 512
                                nc.tensor.matmul(
                                    pm[:, sc * 512:(sc + 1) * 512],
                                    lhsT=lhsT,
                                    rhs=cts[g][:, col:col + 512],
                                    start=(h == 0), stop=(h == 1))
                        k = (t * MG + g) * SP + hg
                        if span_idx % GPS_SPAN_MOD == GPS_SPAN_MOD - 1:
                            ei = etrash_pool.tile([128, SPW], I32, tag="ei",
                                                  name=f"ei{k}")
                            nc.vector.tensor_scalar(
                                ei[:], pm[:], ra_tau_s[:, t:t + 1], SCHRAUDOLPH_B,
                                op0=ALU.mult, op1=ALU.add)
                            erb = etrash_pool.tile([128, SPW], BF16, tag="erb",
                                                   name=f"erb{k}")
                            nc.vector.tensor_scalar(
                                erb[:], ei[:].bitcast(F32), 1.0, None,
                                op0=ALU.mult, op1=ALU.add,
                                accum_out=separts[:, k:k + 1])
                        else:
                            etr = etrash_pool.tile([128, SPW], BF16, tag="etr",
                                                   name=f"etr{k}")
                            nc.scalar.activation(
                                etr[:], pm[:], ACTF.Exp, scale=ra_tau[:, t:t + 1],
                                accum_out=separts[:, k:k + 1])
                        span_idx += 1

            while tasks:
                tasks.popleft()[1]()

            # ---- finalize ----
            nc.vector.reduce_sum(
                sumexp[:],
                separts[:].rearrange("p (t r) -> p t r", t=NT),
                axis=mybir.AxisListType.X)
            nc.scalar.activation(lse[:], sumexp[:], ACTF.Ln)
            nc.vector.tensor_sub(nll_sb[:], lse[:], ltgt[:])
            nc.gpsimd.dma_start(nll_out[:, :], nll_sb[:])

    nc.compile()
    return nc


_CACHE = {}


def _compiled():
    if "nc" not in _CACHE:
        _CACHE["nc"] = build_graph()
    return _CACHE["nc"]


def make_in_maps(anchors, candidates, targets):
    anchors = np.ascontiguousarray(np.asarray(anchors, dtype=np.float32))
    candidates = np.ascontiguousarray(np.asarray(candidates, dtype=np.float32))
    targets = np.asarray(targets, dtype=np.int32)
    tc_full = candidates[targets]          # [N, D] host gather of target rows
    nl = anchors.shape[0] // N_CORES
    in_maps = []
    for c in range(N_CORES):
        sl = slice(c * nl, (c + 1) * nl)
        in_maps.append({
            "anch": np.ascontiguousarray(anchors[sl]),
            "cand": candidates,
            "tcand": np.ascontiguousarray(tc_full[sl]),
        })
    return in_maps


def kernel(anchors, candidates, targets):
    nc = _compiled()
    in_maps = make_in_maps(anchors, candidates, targets)
    res = run_bass_kernel_spmd(nc, in_maps, core_ids=list(range(N_CORES)))
    nll = np.stack([np.asarray(r["nll"], dtype=np.float64) for r in res.results])
    return np.float32(nll.mean())
